# revision 54
# baseline (speedup 1.0000x reference)
"""Trainium2 Bass kernel for DY_Block (EfficientAT DyMN dynamic block).

Data-parallel over batch B=32 across 8 NeuronCores (4 samples/core).

Per core:
  Phase A (batched over the 4 local samples): channel means (DVE f-reduce,
  GPSIMD t-reduce) -> ContextGen joint conv (PE, BN folded) -> hswish ->
  g_c -> routing softmax -> att broadcast -> DyReLU coefs -> per-channel
  branch selection (pick the larger-|slope| affine for the ACT evac; the
  other branch is recovered on DVE as rho*y1+kappa with |rho|<=1) ->
  CoordAtt gate vectors (sigf kept f32 for TS scalar use, sigt bf16).
  Phase B per sample:
    expansion as fp8 DoubleRow matmuls (host ships x in a [40,2,*]
    channel-pair layout; mixed expert weights quantized on the fly) ->
    BN+relu evac (ACT) -> hswish tail: v=min(r,6)/6 (DVE TS), interior
    xe=(r-3)*v written to the zero-padded fp8 conv layout by a GPSIMD STT
    -> depthwise 3x3 as FIVE fp8 DoubleRow matmuls (two taps per pass:
    per-channel diag weights in both j-planes; spatial shifts as AP
    offsets with even j-stride, output cols padded to 128) -> DyReLU:
    y1 via ACT evac with per-channel scale/bias, y2=rho*y1+kappa (DVE TS
    4x), max (DVE TT 2x) -> gt gate (DVE TT 2x, packed broadcast) ->
    projection per f-row so the gf gate folds into the lhsT (wp*gf[c,f],
    built by 4x TS ops), residual added as an identity-matmul into the
    same PSUM accumulation -> BN bias evac (ACT, f32) -> DMA out.
"""
import os
import zlib

import ml_dtypes
import numpy as np

import concourse.bass as bass
import concourse.bacc as bacc
import concourse.tile as tile
from concourse import mybir
from concourse.bass_types import AP
from concourse.bass_utils import run_bass_kernel_spmd

F32 = mybir.dt.float32
BF16 = mybir.dt.bfloat16
F8 = mybir.dt.float8e4
E4M3 = ml_dtypes.float8_e4m3
AX = mybir.AxisListType
OP = mybir.AluOpType
AF = mybir.ActivationFunctionType
DR = mybir.MatmulPerfMode.DoubleRow

B, CIN, CEXP, COUT, F, T = 32, 80, 480, 80, 32, 125
CTX, K, M = 120, 4, 2
TEMP = 30.0
EPS = 1e-3
NCORES = 8
S = B // NCORES          # samples per core
NBLK = CEXP // CTX       # 4 channel blocks of 120
FT = F * T               # 4000
TP = 128                 # padded row stride for depthwise layout
FP = F + 2               # padded f rows
XOFF = 4                 # lead slack
NPAD = XOFF + FP * TP + 4
TOFF = 2                 # t offset inside padded row
NCH = 8                  # chunks per matmul pass
CHW = FT // NCH          # 500 cols per chunk
GF = 4                   # f-rows per depthwise psum group
NG = F // GF             # 8 groups
CP = CIN // 2            # 40 pair-partitions for expansion DoubleRow
MB = 128                 # DoubleRow output block (cols padded 120->128)
# tap order: pairs grouped by dt parity so every pair's j-stride is even
# (odd strides fault the DoubleRow rhs feed on hw)
TAPS = [(-1, -1), (-1, 1), (0, -1), (0, 1), (1, -1), (1, 1),
        (-1, 0), (0, 0), (1, 0)]          # + implicit 10th zero tap
NPAIR = 5
# Source-content tag: changes the HLO signature whenever this file changes so
# the neuronx compile cache (which keys on HLO alone) cannot serve a stale NEFF.
_VTAG = (zlib.crc32(open(__file__, 'rb').read()) % 997) + 2


def _pair_geometry():
    """(base_offset_fn(g), jstride) per DoubleRow tap pair."""
    geo = []
    for p in range(NPAIR):
        df_a, dt_a = TAPS[2 * p]
        if 2 * p + 1 < len(TAPS):
            df_b, dt_b = TAPS[2 * p + 1]
            delta = (df_b - df_a) * TP + (dt_b - dt_a)
        else:
            delta = 0
        assert delta % 2 == 0
        geo.append((df_a, dt_a, delta))
    return geo


PAIR_GEO = _pair_geometry()


def _emit(tc, io, ctx):
    nc = tc.nc

    (xbf, xf8, jlf, jlt, jb3, cvf, cvt, bfb, btb, arw, drw, expw, eb3,
     depw, bnbd, projw, pbias, i10, ident80, idp, identf, vtag, ones1, yout) = io

    wpool = ctx.enter_context(tc.tile_pool(name="weights", bufs=1))
    ctx_pool = ctx.enter_context(tc.tile_pool(name="ctx", bufs=1))
    ps_mm = ctx.enter_context(tc.tile_pool(name="ps_mm", bufs=2, space="PSUM"))
    work = ctx.enter_context(tc.tile_pool(name="work", bufs=2))
    zpool = ctx.enter_context(tc.tile_pool(name="zpool", bufs=1))
    xpool = ctx.enter_context(tc.tile_pool(name="xpool", bufs=2))

    # ---------- persistent weights ----------
    # DMA queue assignment: SP carries the x inputs + phase-A-critical
    # weights (issue order = landing order per queue); the heavy phase-B
    # weight banks ride the DVE/ACT hwdge queues so they don't delay the
    # input-dependent pipeline head.
    def wtile(ap, tag, eng=None):
        t = wpool.tile(list(ap.shape), ap.dtype, tag=tag)
        (eng or nc.sync).dma_start(t[:], ap)
        return t

    xf8_tiles = []
    for s in range(S):
        xfp = xpool.tile([CP, 2 * FT], F8, tag="xf8", bufs=2, name=f"xfp{s}")
        nc.sync.dma_start(xfp[:], xf8[s].rearrange("p a b -> p (a b)"))
        xf8_tiles.append(xfp)
        if s == 1:
            break

    w_jlf = wtile(jlf, "jlf")          # [80,120] f32
    w_jlt = wtile(jlt, "jlt")          # [80,120]
    w_jb3 = wtile(jb3, "jb3")          # [120,1]
    w_arw = wtile(arw, "arw")          # [121,12]
    w_idf = wtile(identf, "idf")       # [128,128] f32 identity
    w_cvf = wtile(cvf, "cvf", nc.scalar)    # [120,480] bf16
    w_cvt = wtile(cvt, "cvt", nc.scalar)    # [120,480] bf16
    w_bfb = wtile(bfb, "bfb", nc.scalar)
    w_btb = wtile(btb, "btb", nc.scalar)
    w_drw = wtile(drw, "drw", nc.scalar)    # [121,1920] bf16
    w_eb3 = wtile(eb3, "eb3", nc.scalar)
    w_bnbd = wtile(bnbd, "bnbd", nc.scalar)
    w_pbias = wtile(pbias, "pbias", nc.scalar)
    w_i80 = wtile(ident80, "i80", nc.scalar)
    w_expw = wtile(expw, "expw", nc.gpsimd)    # [40, K*2*NBLK*128] bf16
    w_depw = wtile(depw, "depw", nc.gpsimd)
    w_projw = wtile(projw, "projw", nc.gpsimd)
    w_i10 = wtile(i10, "i10", nc.gpsimd)

    # ---------- Phase A : batched context ----------
    # channel sums on the PE: accumulating fp8 DoubleRow matmuls against a
    # paired identity (sum over t for cf, over f for ct); evac via ACT
    w_idp = wtile(idp, "idp")          # [40, 2*128] fp8 paired identity
    ga_in = ctx_pool.tile([CIN, S * (F + T)], F32)
    for s in range(S):
        if s < 2:
            xf = xf8_tiles[s]
        else:
            xf = xpool.tile([CP, 2 * FT], F8, tag="xf8", bufs=2)
            nc.sync.dma_start(xf[:], xf8[s].rearrange("p a b -> p (a b)"))
        lhs_i = AP(w_idp[:].tensor, 0, [[2 * MB, CP], [MB, 2], [1, MB]])
        ps_ct = ps_mm.tile([MB, T], F32, tag="pse", bufs=3)
        for f in range(F):
            rhs = AP(xf[:].tensor, f * T, [[2 * FT, CP], [FT, 2], [1, T]])
            nc.tensor.matmul(ps_ct[:], lhs_i, rhs, start=(f == 0),
                             stop=(f == F - 1), perf_mode=DR)
        nc.scalar.activation(ga_in[:, S * F + s * T: S * F + (s + 1) * T],
                             ps_ct[0:CIN, :], AF.Identity)
        ps_cf = ps_mm.tile([MB, F], F32, tag="pse", bufs=3)
        for t in range(T):
            rhs = AP(xf[:].tensor, t, [[2 * FT, CP], [FT, 2], [T, F]])
            nc.tensor.matmul(ps_cf[:], lhs_i, rhs, start=(t == 0),
                             stop=(t == T - 1), perf_mode=DR)
        nc.scalar.activation(ga_in[:, s * F:(s + 1) * F],
                             ps_cf[0:CIN, :], AF.Identity)

    # padded fp8 conv-input ring + matching padded bf16 staging ring: pads
    # zeroed once, interiors rewritten per use; the fp8 copy is produced by a
    # casting SWDGE DMA over the whole flat tile (120 contiguous descriptors)
    xe_ring = []
    xb_ring = []
    for i in range(2):
        xe = zpool.tile([CTX, NPAD], F8, tag=f"xe{i}", name=f"xe{i}")
        xbp = zpool.tile([CTX, NPAD], BF16, tag=f"xbp{i}", name=f"xbp{i}")
        for tl in (xe, xbp):
            nc.vector.memset(tl[:, 0:XOFF + TP], 0.0)
            nc.vector.memset(tl[:, XOFF + (F + 1) * TP:NPAD], 0.0)
            t3 = tl[:, XOFF:XOFF + FP * TP].rearrange("p (f t) -> p f t", t=TP)
            nc.vector.memset(t3[:, 1:1 + F, 0:TOFF], 0.0)
            nc.vector.memset(t3[:, 1:1 + F, TOFF + T:TP], 0.0)
        xe_ring.append(xe)
        xb_ring.append(xbp)

    ps_g1 = ps_mm.tile([CTX, S * F], F32, tag="pse", bufs=3)
    nc.tensor.matmul(ps_g1[:], w_jlf[:], ga_in[:, 0:S * F], start=True, stop=True)
    ps_g2 = ps_mm.tile([CTX, S * T], F32, tag="pse", bufs=3)
    nc.tensor.matmul(ps_g2[:], w_jlt[:], ga_in[:, S * F:], start=True, stop=True)

    r_g = ctx_pool.tile([CTX, S * (F + T)], F32)
    nc.scalar.activation(r_g[:, 0:S * F], ps_g1[:], AF.Relu, bias=w_jb3[:, 0:1], scale=1.0)
    nc.scalar.activation(r_g[:, S * F:], ps_g2[:], AF.Relu, bias=w_jb3[:, 0:1], scale=1.0)
    v_g = ctx_pool.tile([CTX, S * (F + T)], F32)
    nc.vector.tensor_scalar(v_g[:], r_g[:], 6.0, 1.0 / 6.0, OP.min, OP.mult)
    gc_t = r_g  # in-place: (r-3)*v overwrites r
    nc.vector.scalar_tensor_tensor(gc_t[:], r_g[:], -3.0, v_g[:], OP.add, OP.mult)
    gc_b = ctx_pool.tile([CTX, S * (F + T)], BF16)
    nc.vector.tensor_copy(gc_b[:], gc_t[:])

    g_c = ctx_pool.tile([CTX + 1, S], F32)
    tmp_r = ctx_pool.tile([CTX, S], F32)
    nc.vector.tensor_reduce(
        g_c[0:CTX, :], gc_t[:, 0:S * F].rearrange("p (s f) -> p s f", s=S),
        AX.X, OP.add)
    nc.vector.tensor_reduce(
        tmp_r[:], gc_t[:, S * F:].rearrange("p (s t) -> p s t", s=S), AX.X, OP.add)
    nc.vector.tensor_add(g_c[0:CTX, :], g_c[0:CTX, :], tmp_r[:])
    nc.sync.dma_start(g_c[CTX:CTX + 1, :], ones1)

    # routing attention (Exp table first, then everything sigmoid/identity)
    ps_a = ps_mm.tile([S, 3 * K], F32, tag="pse", bufs=3)
    nc.tensor.matmul(ps_a[:], g_c[:], w_arw[:], start=True, stop=True)
    ex_t = ctx_pool.tile([S, 3 * K], F32)
    nc.scalar.activation(ex_t[:], ps_a[:], AF.Exp)
    s3 = ctx_pool.tile([S, 3], F32)
    nc.vector.tensor_reduce(
        s3[:], ex_t[:].rearrange("p (j k) -> p j k", j=3), AX.X, OP.add)
    rec3 = ctx_pool.tile([S, 3], F32)
    nc.vector.reciprocal(rec3[:], s3[:])
    attn = ctx_pool.tile([S, 3 * K], F32)
    for j in range(3):
        nc.vector.tensor_scalar(
            attn[:, j * K:(j + 1) * K], ex_t[:, j * K:(j + 1) * K],
            rec3[:, j:j + 1], None, OP.mult)
    att48 = ctx_pool.tile([S, 3 * K * S], F32)
    nc.vector.tensor_tensor(
        att48[:].rearrange("p (jk s) -> p jk s", s=S),
        attn[:].unsqueeze(2).broadcast_to((S, 3 * K, S)),
        w_idf[0:S, 0:S].unsqueeze(1).broadcast_to((S, 3 * K, S)),
        OP.mult)
    onesS = ctx_pool.tile([S, CTX], F32)
    nc.vector.memset(onesS[:], 1.0)
    ps_ab = ps_mm.tile([CTX, 3 * K * S], F32, tag="pse", bufs=3)
    nc.tensor.matmul(ps_ab[:], onesS[:], att48[:], start=True, stop=True)
    att_b = ctx_pool.tile([CTX, 3 * K * S], F32)
    nc.scalar.copy(att_b[:], ps_ab[:])

    # DyReLU coefficients
    g_cb = ctx_pool.tile([CTX + 1, S], BF16)
    nc.vector.tensor_copy(g_cb[:], g_c[:])
    cj = []
    for j, (sc, of) in enumerate([(2.0, 0.0), (2.0, -1.0), (1.0, -0.5), (1.0, -0.5)]):
        ps_th = ps_mm.tile([S, CEXP], F32, tag="pse", bufs=3)
        nc.tensor.matmul(ps_th[:], g_cb[:], w_drw[:, j * CEXP:(j + 1) * CEXP],
                         start=True, stop=True)
        cft = work.tile([S, CEXP], F32, tag="cft", bufs=2)
        nc.scalar.activation(cft[:], ps_th[:], AF.Sigmoid)
        nc.vector.tensor_scalar(cft[:], cft[:], sc, of, OP.mult, OP.add)
        cj_t = ctx_pool.tile([CTX, NBLK * S], F32, tag=f"cj{j}")
        for blk in range(NBLK):
            ps_c = ps_mm.tile([CTX, S], F32, tag="pse", bufs=3)
            nc.tensor.transpose(
                ps_c[:], cft[:, blk * CTX:(blk + 1) * CTX], w_idf[0:S, 0:S])
            nc.scalar.copy(cj_t[:, blk * S:(blk + 1) * S], ps_c[:])
        cj.append(cj_t)
    for i in range(2):  # fold dep-BN bias: b'_i = a_i*bnb + b_i
        for blk in range(NBLK):
            sl = slice(blk * S, (blk + 1) * S)
            nc.vector.scalar_tensor_tensor(
                cj[2 + i][:, sl], cj[i][:, sl], w_bnbd[:, blk:blk + 1],
                cj[2 + i][:, sl], OP.mult, OP.add)

    # per-channel big-branch selection: y1 = aB*e + bB on ACT; y2 = rho*y1+kappa
    NS = NBLK * S
    def ct_tile(tag):
        return ctx_pool.tile([CTX, NS], F32, tag=tag, name=tag)
    ab1 = ct_tile("ab1"); ab2 = ct_tile("ab2"); gsel = ct_tile("gsel")
    aB = ct_tile("aB"); bB = ct_tile("bB")
    rho = ct_tile("rho"); kap = ct_tile("kap")
    tA = ct_tile("tA"); tB = ct_tile("tB")
    nc.vector.tensor_scalar(ab1[:], cj[0][:], -1.0, None, OP.mult)
    nc.vector.tensor_tensor(ab1[:], cj[0][:], ab1[:], OP.max)
    nc.vector.tensor_scalar(ab2[:], cj[1][:], -1.0, None, OP.mult)
    nc.vector.tensor_tensor(ab2[:], cj[1][:], ab2[:], OP.max)
    nc.vector.tensor_tensor(gsel[:], ab1[:], ab2[:], OP.is_ge)
    nc.vector.tensor_tensor(tA[:], cj[0][:], cj[1][:], OP.subtract)
    nc.vector.tensor_tensor(tA[:], gsel[:], tA[:], OP.mult)
    nc.vector.tensor_tensor(aB[:], cj[1][:], tA[:], OP.add)
    nc.vector.tensor_tensor(tB[:], cj[2][:], cj[3][:], OP.subtract)
    nc.vector.tensor_tensor(tB[:], gsel[:], tB[:], OP.mult)
    nc.vector.tensor_tensor(bB[:], cj[3][:], tB[:], OP.add)
    nc.vector.tensor_tensor(tA[:], cj[0][:], cj[1][:], OP.add)
    nc.vector.tensor_tensor(tA[:], tA[:], aB[:], OP.subtract)   # a_small
    nc.vector.tensor_tensor(tB[:], cj[2][:], cj[3][:], OP.add)
    nc.vector.tensor_tensor(tB[:], tB[:], bB[:], OP.subtract)   # b_small
    nc.vector.reciprocal(kap[:], aB[:])
    nc.vector.tensor_tensor(rho[:], tA[:], kap[:], OP.mult)     # rho = aS/aB
    nc.vector.tensor_tensor(tA[:], rho[:], bB[:], OP.mult)
    nc.vector.tensor_tensor(kap[:], tB[:], tA[:], OP.subtract)  # kappa

    # CoordAtt gates: sigf f32 (TS scalar source), sigt bf16 (packed TT)
    sigf = ctx_pool.tile([CTX, NBLK * S * F], F32)
    sigt = ctx_pool.tile([CTX, NBLK * S * T], BF16)
    for blk in range(NBLK):
        ps_f = ps_mm.tile([CTX, S * F], F32, tag="pse", bufs=3)
        nc.tensor.matmul(ps_f[:], w_cvf[:, blk * CTX:(blk + 1) * CTX],
                         gc_b[:, 0:S * F], start=True, stop=True)
        nc.scalar.activation(sigf[:, blk * S * F:(blk + 1) * S * F], ps_f[:],
                             AF.Sigmoid, bias=w_bfb[:, blk:blk + 1], scale=1.0)
        ps_t2 = ps_mm.tile([CTX, S * T], F32, tag="pse", bufs=3)
        nc.tensor.matmul(ps_t2[:], w_cvt[:, blk * CTX:(blk + 1) * CTX],
                         gc_b[:, S * F:], start=True, stop=True)
        nc.scalar.activation(sigt[:, blk * S * T:(blk + 1) * S * T], ps_t2[:],
                             AF.Sigmoid, bias=w_btb[:, blk:blk + 1], scale=1.0)

    # ---------- Phase B : per-sample heavy pipeline ----------
    JW = 2 * NBLK * MB          # 1024: we row length (j, blk, i)

    for s in range(S):
        x_f8 = xpool.tile([CP, 2 * FT], F8, tag="xf8", bufs=2)
        nc.sync.dma_start(x_f8[:], xf8[s].rearrange("p a b -> p (a b)"))

        def acol(jr, k, parts):
            c0 = (jr * K + k) * S + s
            return att_b[0:parts, c0:c0 + 1]

        # --- mix expert weights ---
        we = work.tile([CP, JW], BF16, tag="we")
        nc.vector.tensor_scalar(we[:], w_expw[:, 0:JW], acol(0, 0, CP), None, OP.mult)
        for k in range(1, K - 1):
            nc.vector.scalar_tensor_tensor(
                we[:], w_expw[:, k * JW:(k + 1) * JW], acol(0, k, CP), we[:],
                OP.mult, OP.add)
        wef = work.tile([CP, JW], F8, tag="wef")
        nc.vector.scalar_tensor_tensor(
            wef[:], w_expw[:, (K - 1) * JW:K * JW], acol(0, K - 1, CP), we[:],
            OP.mult, OP.add)

        wd = work.tile([CTX, NBLK * 10], F32, tag="wd")
        nc.vector.tensor_scalar(wd[:], w_depw[:, 0:NBLK * 10], acol(1, 0, CTX),
                                None, OP.mult)
        for k in range(1, K):
            nc.vector.scalar_tensor_tensor(
                wd[:], w_depw[:, k * NBLK * 10:(k + 1) * NBLK * 10],
                acol(1, k, CTX), wd[:], OP.mult, OP.add)

        wp = work.tile([CTX, NBLK * COUT], BF16, tag="wp")
        nc.vector.tensor_scalar(wp[:], w_projw[:, 0:NBLK * COUT], acol(2, 0, CTX),
                                None, OP.mult)
        for k in range(1, K):
            nc.vector.scalar_tensor_tensor(
                wp[:], w_projw[:, k * NBLK * COUT:(k + 1) * NBLK * COUT],
                acol(2, k, CTX), wp[:], OP.mult, OP.add)

        zs = []
        for blk in range(NBLK):
            bs = blk * S + s
            # --- expansion: fp8 DoubleRow over channel pairs ---
            r_blk = work.tile([CTX, FT], BF16, tag="rb", bufs=2)
            for ch in range(NCH):
                ps_e = ps_mm.tile([MB, CHW], F32, tag="pse", bufs=3)
                lhs = AP(wef[:].tensor, blk * MB,
                         [[JW, CP], [NBLK * MB, 2], [1, MB]])
                rhs = AP(x_f8[:].tensor, ch * CHW,
                         [[2 * FT, CP], [FT, 2], [1, CHW]])
                nc.tensor.matmul(ps_e[:], lhs, rhs, start=True, stop=True,
                                 perf_mode=DR)
                nc.scalar.activation(r_blk[:, ch * CHW:(ch + 1) * CHW],
                                     ps_e[0:CTX, :], AF.Relu,
                                     bias=w_eb3[:, blk:blk + 1], scale=1.0)
            # --- hswish tail: bf16 on DVE, fp8 quantize via casting SWDGE DMA ---
            v_blk = work.tile([CTX, FT], BF16, tag="vb", bufs=2)
            nc.vector.tensor_scalar(v_blk[:], r_blk[:], 6.0, 1.0 / 6.0,
                                    OP.min, OP.mult)
            xe = xe_ring[bs % 2]
            xbp = xb_ring[bs % 2]
            xb3 = xbp[:, XOFF:XOFF + FP * TP].rearrange("p (f t) -> p f t", t=TP)
            xbi = xb3[:, 1:1 + F, TOFF:TOFF + T]
            nc.vector.tensor_scalar(
                xbi, r_blk[:].rearrange("p (f t) -> p f t", t=T),
                -3.0, None, OP.add)
            nc.vector.tensor_tensor(
                xbi, xbi, v_blk[:].rearrange("p (f t) -> p f t", t=T), OP.mult)
            nc.gpsimd.dma_start(xe[:], xbp[:])

            # --- diag weights for all 10 taps in one TT (GPSIMD) ---
            dg = work.tile([CTX, 10 * MB], F8, tag="dg", bufs=3)
            nc.gpsimd.tensor_tensor(
                dg[:].rearrange("p (t i) -> p t i", i=MB),
                w_i10[:].rearrange("p (t i) -> p t i", i=MB),
                wd[:, blk * 10:(blk + 1) * 10].unsqueeze(2)
                   .broadcast_to((CTX, 10, MB)),
                OP.mult)

            # --- depthwise: 5 fp8 DoubleRow passes per f-group ---
            z = zpool.tile([CTX, FT], BF16, tag=f"z{blk}")
            xe_pitch = xe[:].ap[0][0]
            dg_pitch = dg[:].ap[0][0]
            for g in range(NG):
                ps_d = ps_mm.tile([MB, GF * T], F32, tag="psd", bufs=3)
                for p, (df_a, dt_a, delta) in enumerate(PAIR_GEO):
                    base = XOFF + (1 + GF * g + df_a) * TP + TOFF + dt_a
                    rhs = AP(xe[:].tensor, base,
                             [[xe_pitch, CTX], [delta, 2], [TP, GF], [1, T]])
                    lhs = AP(dg[:].tensor, 2 * p * MB,
                             [[dg_pitch, CTX], [MB, 2], [1, MB]])
                    out_ap = AP(ps_d[:].tensor, 0,
                                [[ps_d[:].ap[0][0], MB], [T, GF], [1, T]])
                    nc.tensor.matmul(out_ap, lhs, rhs, start=(p == 0),
                                     stop=(p == NPAIR - 1), perf_mode=DR)
                # DyReLU: y1 on ACT (big branch), y2 = rho*y1+kappa on DVE
                sl = slice(g * GF * T, (g + 1) * GF * T)
                y1 = work.tile([CTX, GF * T], BF16, tag="y1", bufs=4)
                nc.scalar.activation(y1[:], ps_d[0:CTX, :], AF.Identity,
                                     bias=bB[:, bs:bs + 1],
                                     scale=aB[:, bs:bs + 1])
                y2 = work.tile([CTX, GF * T], BF16, tag="y2", bufs=4)
                nc.vector.tensor_scalar(y2[:], y1[:], rho[:, bs:bs + 1],
                                        kap[:, bs:bs + 1], OP.mult, OP.add)
                nc.vector.tensor_tensor(y1[:], y1[:], y2[:], OP.max)
                gt_v = sigt[:, bs * T:(bs + 1) * T].unsqueeze(1) \
                    .broadcast_to((CTX, GF, T))
                eng = nc.vector if (bs + g) % 2 == 0 else nc.gpsimd
                eng.tensor_tensor(
                    z[:, sl].rearrange("p (f t) -> p f t", t=T),
                    y1[:].rearrange("p (f t) -> p f t", t=T),
                    gt_v, OP.mult)
            zs.append(z)

        # --- projection per f-row (gf folded into lhsT) + residual ---

        wpgfs = []
        for blk in range(NBLK):
            # wpgf[c, (f, o)] = wp[c, o] * sigf[c, f]  (one 1x TT per blk)
            wpgf = work.tile([CTX, F * COUT], BF16, tag=f"wpgf{blk}", bufs=1)
            nc.vector.tensor_tensor(
                wpgf[:].rearrange("p (f o) -> p f o", o=COUT),
                wp[:, blk * COUT:(blk + 1) * COUT].unsqueeze(1)
                  .broadcast_to((CTX, F, COUT)),
                sigf[:, (blk * S + s) * F:(blk * S + s + 1) * F].unsqueeze(2)
                  .broadcast_to((CTX, F, COUT)),
                OP.mult)
            wpgfs.append(wpgf)
        xr = xpool.tile([CIN, FT], BF16, tag="xbf", bufs=2)
        nc.sync.dma_start(xr[:], xbf[s])
        for g2 in range(NG):
            ps_p = ps_mm.tile([COUT, GF * T], F32, tag="psp", bufs=2)
            for fi in range(GF):
                f = g2 * GF + fi
                for blk in range(NBLK):
                    nc.tensor.matmul(
                        ps_p[:, fi * T:(fi + 1) * T],
                        wpgfs[blk][:, f * COUT:(f + 1) * COUT],
                        zs[blk][:, f * T:(f + 1) * T],
                        start=(blk == 0), stop=False)
                nc.tensor.matmul(
                    ps_p[:, fi * T:(fi + 1) * T], w_i80[:],
                    xr[:, f * T:(f + 1) * T], start=False, stop=True)
            outs = work.tile([COUT, GF * T], F32, tag="outs", bufs=2)
            nc.scalar.activation(outs[:], ps_p[:], AF.Identity,
                                 bias=w_pbias[:, 0:1], scale=1.0)
            nc.sync.dma_start(yout[s, :, g2 * GF * T:(g2 + 1) * GF * T], outs[:])


def _host_prep(inputs):
    """Precompute packed/folded weight arrays (numpy, O(weights))."""
    p = {k: np.asarray(v, dtype=np.float32) for k, v in inputs.items()}
    inv_j = p["cg_joint_gamma"] / np.sqrt(p["cg_joint_var"] + EPS)
    sh_j = p["cg_joint_beta"] - p["cg_joint_mean"] * inv_j
    jlf = (p["cg_joint_w"].T * inv_j[None, :]) / T
    jlt = (p["cg_joint_w"].T * inv_j[None, :]) / F
    jb3 = (sh_j + 3.0)[:, None]

    cvf = np.ascontiguousarray(p["cg_convf_w"].T).astype(ml_dtypes.bfloat16)
    cvt = np.ascontiguousarray(p["cg_convt_w"].T).astype(ml_dtypes.bfloat16)
    bfb = np.ascontiguousarray(p["cg_convf_b"].reshape(NBLK, CTX).T)
    btb = np.ascontiguousarray(p["cg_convt_b"].reshape(NBLK, CTX).T)

    sc = 1.0 / ((F + T) * TEMP)
    arw0 = np.concatenate([p["exp_res_w"], p["dep_res_w"], p["proj_res_w"]], 0).T * sc
    arb0 = np.concatenate([p["exp_res_b"], p["dep_res_b"], p["proj_res_b"]]) / TEMP
    arw = np.ascontiguousarray(np.vstack([arw0, arb0[None, :]]))

    drw_r = p["dr_w"].reshape(CEXP, 2 * M, CTX).transpose(1, 0, 2)
    drw0 = drw_r.reshape(2 * M * CEXP, CTX).T / (F + T)
    drb_r = p["dr_b"].reshape(CEXP, 2 * M).T.reshape(-1)
    drw = np.ascontiguousarray(np.vstack([drw0, drb_r[None, :]])).astype(ml_dtypes.bfloat16)

    # expansion weights: BN-folded, padded to 128-col blocks, channel-pair
    # layout [40, K, 2, NBLK, 128]
    inv_e = p["exp_bn_gamma"] / np.sqrt(p["exp_bn_var"] + EPS)
    sh_e = p["exp_bn_beta"] - p["exp_bn_mean"] * inv_e
    ew = (p["exp_weight"] * inv_e[None, :, None]).transpose(0, 2, 1)  # [K,80,480]
    ewp = np.zeros((K, CIN, NBLK, MB), np.float32)
    ewp[:, :, :, 0:CTX] = ew.reshape(K, CIN, NBLK, CTX)
    expw = np.zeros((CP, K, 2, NBLK, MB), np.float32)
    for j in range(2):
        expw[:, :, j] = ewp[:, j * CP:(j + 1) * CP].transpose(1, 0, 2, 3)
    expw = np.ascontiguousarray(
        expw.reshape(CP, K * 2 * NBLK * MB)).astype(ml_dtypes.bfloat16)
    eb3 = np.ascontiguousarray((sh_e + 3.0).reshape(NBLK, CTX).T)

    # depthwise: BN-folded per-channel 3x3 in dt-parity tap order + zero pad
    inv_d = p["dep_bn_gamma"] / np.sqrt(p["dep_bn_var"] + EPS)
    sh_d = p["dep_bn_beta"] - p["dep_bn_mean"] * inv_d
    dw = (p["dep_weight"] * inv_d[None, :, None, None])  # [K, CEXP, 3, 3]
    dw10 = np.zeros((K, CEXP, 10), np.float32)
    for t, (df, dt) in enumerate(TAPS):
        dw10[:, :, t] = dw[:, :, df + 1, dt + 1]
    dw_b = dw10.reshape(K, NBLK, CTX, 10).transpose(2, 0, 1, 3)
    depw = np.ascontiguousarray(dw_b.reshape(CTX, K * NBLK * 10))
    bnbd = np.ascontiguousarray(sh_d.reshape(NBLK, CTX).T)

    inv_p = p["proj_bn_gamma"] / np.sqrt(p["proj_bn_var"] + EPS)
    sh_p = p["proj_bn_beta"] - p["proj_bn_mean"] * inv_p
    pw = p["proj_weight"] * inv_p[None, :, None]        # [K, 80, 480]
    pw_b = pw.transpose(2, 0, 1).reshape(NBLK, CTX, K, COUT).transpose(1, 2, 0, 3)
    projw = np.ascontiguousarray(
        pw_b.reshape(CTX, K * NBLK * COUT)).astype(ml_dtypes.bfloat16)
    pbias = sh_p[:, None]

    # 10 fp8 identity planes [120, 10*128] (plane t: delta_{i,c}; plane 9 zero)
    i10 = np.zeros((CTX, 10, MB), np.float32)
    for t in range(9):
        i10[np.arange(CTX), t, np.arange(CTX)] = 1.0
    i10 = i10.reshape(CTX, 10 * MB).astype(E4M3)

    identf = np.eye(128, dtype=np.float32)
    ident80 = np.eye(80, dtype=np.float32).astype(ml_dtypes.bfloat16)
    idp = np.zeros((CP, 2, MB), np.float32)
    for p_ in range(CP):
        for j_ in range(2):
            idp[p_, j_, p_ + CP * j_] = 1.0
    idp = idp.reshape(CP, 2 * MB).astype(E4M3)
    return dict(jlf=jlf, jlt=jlt, jb3=jb3, cvf=cvf, cvt=cvt, bfb=bfb, btb=btb,
                arw=arw, drw=drw, expw=expw, eb3=eb3,
                depw=depw, bnbd=bnbd, projw=projw, pbias=pbias,
                i10=i10, ident80=ident80, idp=idp, identf=identf,
                vtag=np.zeros((1, _VTAG), np.float32),
                ones1=np.ones((1, S), np.float32))


_BUILT = {}


def _build():
    if "nc" in _BUILT:
        return _BUILT["nc"]
    nc = bacc.Bacc("TRN2", target_bir_lowering=False, debug=False,
                   num_devices=NCORES)
    d = lambda n, s, dt=F32: nc.dram_tensor(n, list(s), dt, kind="ExternalInput").ap()
    io = [
        d("xbf", (S, CIN, FT), BF16),
        d("xf8", (S, CP, 2, FT), F8),
        d("jlf", (CIN, CTX)), d("jlt", (CIN, CTX)), d("jb3", (CTX, 1)),
        d("cvf", (CTX, CEXP), BF16), d("cvt", (CTX, CEXP), BF16),
        d("bfb", (CTX, NBLK)), d("btb", (CTX, NBLK)),
        d("arw", (CTX + 1, 3 * K)),
        d("drw", (CTX + 1, 2 * M * CEXP), BF16),
        d("expw", (CP, K * 2 * NBLK * MB), BF16), d("eb3", (CTX, NBLK)),
        d("depw", (CTX, K * NBLK * 10)), d("bnbd", (CTX, NBLK)),
        d("projw", (CTX, K * NBLK * COUT), BF16), d("pbias", (COUT, 1)),
        d("i10", (CTX, 10 * MB), F8),
        d("ident80", (COUT, COUT), BF16),
        d("idp", (CP, 2 * MB), F8),
        d("identf", (128, 128)), d("vtag", (1, _VTAG)),
        d("ones1", (1, S)),
        nc.dram_tensor("y", [S, COUT, FT], F32, kind="ExternalOutput").ap(),
    ]
    from contextlib import ExitStack
    with tile.TileContext(nc) as tc:
        with ExitStack() as es:
            _emit(tc, io, es)
    nc.compile()
    _BUILT["nc"] = nc
    return nc


def _purge_stale_neff_cache():
    """The neuronx compile cache can key on the HLO signature alone; purge
    defensively so a stale NEFF can never be loaded."""
    import shutil
    base = os.path.expanduser("~/.neuron-compile-cache")
    tag = os.path.join(base, f".dyblock_vtag_{_VTAG}")
    if os.path.exists(base) and not os.path.exists(tag):
        shutil.rmtree(base, ignore_errors=True)
        os.makedirs(base, exist_ok=True)
        open(tag, "w").close()


def make_in_maps(inputs):
    host = _host_prep(inputs)
    x = np.asarray(inputs["x"], dtype=np.float32).reshape(B, CIN, FT)
    xbf = x.astype(ml_dtypes.bfloat16)
    xf8 = np.stack([x[:, 0:CP], x[:, CP:CIN]], axis=2).astype(E4M3)  # [B,40,2,FT]
    in_maps = []
    for c in range(NCORES):
        m = {"xbf": np.ascontiguousarray(xbf[c * S:(c + 1) * S]),
             "xf8": np.ascontiguousarray(xf8[c * S:(c + 1) * S])}
        m.update(host)
        in_maps.append(m)
    return in_maps


def assemble_out(ys):
    out = np.concatenate(list(ys), axis=0)                # [B, COUT, FT]
    return np.ascontiguousarray(out.reshape(B, COUT, F, T))


def kernel(**inputs):
    _purge_stale_neff_cache()
    nc = _build()
    in_maps = make_in_maps(inputs)
    res = run_bass_kernel_spmd(nc, in_maps, list(range(NCORES)))
    return assemble_out([res.results[c]["y"] for c in range(NCORES)])


if __name__ == "__main__":
    import reference as ref
    inp = {k: np.asarray(v) for k, v in ref.setup_inputs().items()}
    got = kernel(**inp)
    from np_ref import forward_np
    exp = forward_np(inp)
    rel = np.abs(got - exp).max() / np.abs(exp).max()
    print("rel err vs np_ref:", rel)


# revision 55
# speedup vs baseline: 1.0120x; 1.0120x over previous
"""Trainium2 Bass kernel for DY_Block (EfficientAT DyMN dynamic block).

Data-parallel over batch B=32 across 8 NeuronCores (4 samples/core).

Per core:
  Phase A (batched over the 4 local samples): channel means (DVE f-reduce,
  GPSIMD t-reduce) -> ContextGen joint conv (PE, BN folded) -> hswish ->
  g_c -> routing softmax -> att broadcast -> DyReLU coefs -> per-channel
  branch selection (pick the larger-|slope| affine for the ACT evac; the
  other branch is recovered on DVE as rho*y1+kappa with |rho|<=1) ->
  CoordAtt gate vectors (sigf kept f32 for TS scalar use, sigt bf16).
  Phase B per sample:
    expansion as fp8 DoubleRow matmuls (host ships x in a [40,2,*]
    channel-pair layout; mixed expert weights quantized on the fly) ->
    BN+relu evac (ACT) -> hswish tail: v=min(r,6)/6 (DVE TS), interior
    xe=(r-3)*v written to the zero-padded fp8 conv layout by a GPSIMD STT
    -> depthwise 3x3 as FIVE fp8 DoubleRow matmuls (two taps per pass:
    per-channel diag weights in both j-planes; spatial shifts as AP
    offsets with even j-stride, output cols padded to 128) -> DyReLU:
    y1 via ACT evac with per-channel scale/bias, y2=rho*y1+kappa (DVE TS
    4x), max (DVE TT 2x) -> gt gate (DVE TT 2x, packed broadcast) ->
    projection per f-row so the gf gate folds into the lhsT (wp*gf[c,f],
    built by 4x TS ops), residual added as an identity-matmul into the
    same PSUM accumulation -> BN bias evac (ACT, f32) -> DMA out.
"""
import os
import zlib

import ml_dtypes
import numpy as np

import concourse.bass as bass
import concourse.bacc as bacc
import concourse.tile as tile
from concourse import mybir
from concourse.bass_types import AP
from concourse.bass_utils import run_bass_kernel_spmd

F32 = mybir.dt.float32
BF16 = mybir.dt.bfloat16
F8 = mybir.dt.float8e4
E4M3 = ml_dtypes.float8_e4m3
AX = mybir.AxisListType
OP = mybir.AluOpType
AF = mybir.ActivationFunctionType
DR = mybir.MatmulPerfMode.DoubleRow

B, CIN, CEXP, COUT, F, T = 32, 80, 480, 80, 32, 125
CTX, K, M = 120, 4, 2
TEMP = 30.0
EPS = 1e-3
NCORES = 8
S = B // NCORES          # samples per core
NBLK = CEXP // CTX       # 4 channel blocks of 120
FT = F * T               # 4000
TP = 128                 # padded row stride for depthwise layout
FP = F + 2               # padded f rows
XOFF = 4                 # lead slack
NPAD = XOFF + FP * TP + 4
TOFF = 2                 # t offset inside padded row
NCH = 8                  # chunks per matmul pass
CHW = FT // NCH          # 500 cols per chunk
GF = 4                   # f-rows per depthwise psum group
NG = F // GF             # 8 groups
CP = CIN // 2            # 40 pair-partitions for expansion DoubleRow
MB = 128                 # DoubleRow output block (cols padded 120->128)
# tap order: pairs grouped by dt parity so every pair's j-stride is even
# (odd strides fault the DoubleRow rhs feed on hw)
TAPS = [(-1, -1), (-1, 1), (0, -1), (0, 1), (1, -1), (1, 1),
        (-1, 0), (0, 0), (1, 0)]          # + implicit 10th zero tap
NPAIR = 5
# Source-content tag: changes the HLO signature whenever this file changes so
# the neuronx compile cache (which keys on HLO alone) cannot serve a stale NEFF.
_VTAG = (zlib.crc32(open(__file__, 'rb').read()) % 997) + 2


def _pair_geometry():
    """(base_offset_fn(g), jstride) per DoubleRow tap pair."""
    geo = []
    for p in range(NPAIR):
        df_a, dt_a = TAPS[2 * p]
        if 2 * p + 1 < len(TAPS):
            df_b, dt_b = TAPS[2 * p + 1]
            delta = (df_b - df_a) * TP + (dt_b - dt_a)
        else:
            delta = 0
        assert delta % 2 == 0
        geo.append((df_a, dt_a, delta))
    return geo


PAIR_GEO = _pair_geometry()


def _emit(tc, io, ctx):
    nc = tc.nc

    (xbf, xf8, jlf, jlt, jb3, cvf, cvt, bfb, btb, arw, drw, expw, eb3,
     depw, bnbd, projw, pbias, i10, ident80, idp, identf, vtag, ones1, yout) = io

    wpool = ctx.enter_context(tc.tile_pool(name="weights", bufs=1))
    ctx_pool = ctx.enter_context(tc.tile_pool(name="ctx", bufs=1))
    ps_mm = ctx.enter_context(tc.tile_pool(name="ps_mm", bufs=2, space="PSUM"))
    work = ctx.enter_context(tc.tile_pool(name="work", bufs=2))
    zpool = ctx.enter_context(tc.tile_pool(name="zpool", bufs=1))
    xpool = ctx.enter_context(tc.tile_pool(name="xpool", bufs=2))

    # ---------- persistent weights ----------
    # DMA queue assignment: SP carries the x inputs + phase-A-critical
    # weights (issue order = landing order per queue); the heavy phase-B
    # weight banks ride the DVE/ACT hwdge queues so they don't delay the
    # input-dependent pipeline head.
    def wtile(ap, tag, eng=None):
        t = wpool.tile(list(ap.shape), ap.dtype, tag=tag)
        (eng or nc.sync).dma_start(t[:], ap)
        return t

    w_idp = wtile(idp, "idp")          # [40, 2*128] fp8 paired identity
    xf8_tiles = []
    for s in range(S):
        xfp = xpool.tile([CP, 2 * FT], F8, tag="xf8", bufs=2, name=f"xfp{s}")
        nc.sync.dma_start(xfp[:], xf8[s].rearrange("p a b -> p (a b)"))
        xf8_tiles.append(xfp)
        if s == 1:
            break

    w_jlf = wtile(jlf, "jlf")          # [80,120] f32
    w_jlt = wtile(jlt, "jlt")          # [80,120]
    w_jb3 = wtile(jb3, "jb3")          # [120,1]
    w_arw = wtile(arw, "arw")          # [121,12]
    w_idf = wtile(identf, "idf")       # [128,128] f32 identity
    w_cvf = wtile(cvf, "cvf", nc.scalar)    # [120,480] bf16
    w_cvt = wtile(cvt, "cvt", nc.scalar)    # [120,480] bf16
    w_bfb = wtile(bfb, "bfb", nc.scalar)
    w_btb = wtile(btb, "btb", nc.scalar)
    w_drw = wtile(drw, "drw", nc.scalar)    # [121,1920] bf16
    w_eb3 = wtile(eb3, "eb3", nc.scalar)
    w_bnbd = wtile(bnbd, "bnbd", nc.scalar)
    w_pbias = wtile(pbias, "pbias", nc.scalar)
    w_i80 = wtile(ident80, "i80", nc.scalar)
    w_expw = wtile(expw, "expw", nc.gpsimd)    # [40, K*2*NBLK*128] bf16
    w_depw = wtile(depw, "depw", nc.gpsimd)
    w_projw = wtile(projw, "projw", nc.gpsimd)
    w_i10 = wtile(i10, "i10", nc.gpsimd)

    # ---------- Phase A : batched context ----------
    # channel sums on the PE: accumulating fp8 DoubleRow matmuls against a
    # paired identity (sum over t for cf, over f for ct); evac via ACT
    ga_in = ctx_pool.tile([CIN, S * (F + T)], F32)
    for s in range(S):
        if s < 2:
            xf = xf8_tiles[s]
        else:
            xf = xpool.tile([CP, 2 * FT], F8, tag="xf8", bufs=2)
            nc.sync.dma_start(xf[:], xf8[s].rearrange("p a b -> p (a b)"))
        lhs_i = AP(w_idp[:].tensor, 0, [[2 * MB, CP], [MB, 2], [1, MB]])
        ps_ct = ps_mm.tile([MB, T], F32, tag="pse", bufs=3)
        for f in range(F):
            rhs = AP(xf[:].tensor, f * T, [[2 * FT, CP], [FT, 2], [1, T]])
            nc.tensor.matmul(ps_ct[:], lhs_i, rhs, start=(f == 0),
                             stop=(f == F - 1), perf_mode=DR)
        nc.scalar.activation(ga_in[:, S * F + s * T: S * F + (s + 1) * T],
                             ps_ct[0:CIN, :], AF.Identity)
        ps_cf = ps_mm.tile([MB, F], F32, tag="pse", bufs=3)
        for t in range(T):
            rhs = AP(xf[:].tensor, t, [[2 * FT, CP], [FT, 2], [T, F]])
            nc.tensor.matmul(ps_cf[:], lhs_i, rhs, start=(t == 0),
                             stop=(t == T - 1), perf_mode=DR)
        nc.scalar.activation(ga_in[:, s * F:(s + 1) * F],
                             ps_cf[0:CIN, :], AF.Identity)

    # padded fp8 conv-input ring + matching padded bf16 staging ring: pads
    # zeroed once, interiors rewritten per use; the fp8 copy is produced by a
    # casting SWDGE DMA over the whole flat tile (120 contiguous descriptors)
    xe_ring = []
    xb_ring = []
    for i in range(2):
        xe = zpool.tile([CTX, NPAD], F8, tag=f"xe{i}", name=f"xe{i}")
        xbp = zpool.tile([CTX, NPAD], BF16, tag=f"xbp{i}", name=f"xbp{i}")
        for tl in (xe, xbp):
            nc.vector.memset(tl[:, 0:XOFF + TP], 0.0)
            nc.vector.memset(tl[:, XOFF + (F + 1) * TP:NPAD], 0.0)
            t3 = tl[:, XOFF:XOFF + FP * TP].rearrange("p (f t) -> p f t", t=TP)
            nc.vector.memset(t3[:, 1:1 + F, 0:TOFF], 0.0)
            nc.vector.memset(t3[:, 1:1 + F, TOFF + T:TP], 0.0)
        xe_ring.append(xe)
        xb_ring.append(xbp)

    ps_g1 = ps_mm.tile([CTX, S * F], F32, tag="pse", bufs=3)
    nc.tensor.matmul(ps_g1[:], w_jlf[:], ga_in[:, 0:S * F], start=True, stop=True)
    ps_g2 = ps_mm.tile([CTX, S * T], F32, tag="pse", bufs=3)
    nc.tensor.matmul(ps_g2[:], w_jlt[:], ga_in[:, S * F:], start=True, stop=True)

    r_g = ctx_pool.tile([CTX, S * (F + T)], F32)
    nc.scalar.activation(r_g[:, 0:S * F], ps_g1[:], AF.Relu, bias=w_jb3[:, 0:1], scale=1.0)
    nc.scalar.activation(r_g[:, S * F:], ps_g2[:], AF.Relu, bias=w_jb3[:, 0:1], scale=1.0)
    v_g = ctx_pool.tile([CTX, S * (F + T)], F32)
    nc.vector.tensor_scalar(v_g[:], r_g[:], 6.0, 1.0 / 6.0, OP.min, OP.mult)
    gc_t = r_g  # in-place: (r-3)*v overwrites r
    nc.vector.scalar_tensor_tensor(gc_t[:], r_g[:], -3.0, v_g[:], OP.add, OP.mult)
    gc_b = ctx_pool.tile([CTX, S * (F + T)], BF16)
    nc.vector.tensor_copy(gc_b[:], gc_t[:])

    g_c = ctx_pool.tile([CTX + 1, S], F32)
    tmp_r = ctx_pool.tile([CTX, S], F32)
    nc.vector.tensor_reduce(
        g_c[0:CTX, :], gc_t[:, 0:S * F].rearrange("p (s f) -> p s f", s=S),
        AX.X, OP.add)
    nc.vector.tensor_reduce(
        tmp_r[:], gc_t[:, S * F:].rearrange("p (s t) -> p s t", s=S), AX.X, OP.add)
    nc.vector.tensor_add(g_c[0:CTX, :], g_c[0:CTX, :], tmp_r[:])
    nc.sync.dma_start(g_c[CTX:CTX + 1, :], ones1)

    # routing attention (Exp table first, then everything sigmoid/identity)
    ps_a = ps_mm.tile([S, 3 * K], F32, tag="pse", bufs=3)
    nc.tensor.matmul(ps_a[:], g_c[:], w_arw[:], start=True, stop=True)
    ex_t = ctx_pool.tile([S, 3 * K], F32)
    nc.scalar.activation(ex_t[:], ps_a[:], AF.Exp)
    s3 = ctx_pool.tile([S, 3], F32)
    nc.vector.tensor_reduce(
        s3[:], ex_t[:].rearrange("p (j k) -> p j k", j=3), AX.X, OP.add)
    rec3 = ctx_pool.tile([S, 3], F32)
    nc.vector.reciprocal(rec3[:], s3[:])
    attn = ctx_pool.tile([S, 3 * K], F32)
    for j in range(3):
        nc.vector.tensor_scalar(
            attn[:, j * K:(j + 1) * K], ex_t[:, j * K:(j + 1) * K],
            rec3[:, j:j + 1], None, OP.mult)
    att48 = ctx_pool.tile([S, 3 * K * S], F32)
    nc.vector.tensor_tensor(
        att48[:].rearrange("p (jk s) -> p jk s", s=S),
        attn[:].unsqueeze(2).broadcast_to((S, 3 * K, S)),
        w_idf[0:S, 0:S].unsqueeze(1).broadcast_to((S, 3 * K, S)),
        OP.mult)
    onesS = ctx_pool.tile([S, CTX], F32)
    nc.vector.memset(onesS[:], 1.0)
    ps_ab = ps_mm.tile([CTX, 3 * K * S], F32, tag="pse", bufs=3)
    nc.tensor.matmul(ps_ab[:], onesS[:], att48[:], start=True, stop=True)
    att_b = ctx_pool.tile([CTX, 3 * K * S], F32)
    nc.scalar.copy(att_b[:], ps_ab[:])

    # DyReLU coefficients
    g_cb = ctx_pool.tile([CTX + 1, S], BF16)
    nc.vector.tensor_copy(g_cb[:], g_c[:])
    cj = []
    for j, (sc, of) in enumerate([(2.0, 0.0), (2.0, -1.0), (1.0, -0.5), (1.0, -0.5)]):
        ps_th = ps_mm.tile([S, CEXP], F32, tag="pse", bufs=3)
        nc.tensor.matmul(ps_th[:], g_cb[:], w_drw[:, j * CEXP:(j + 1) * CEXP],
                         start=True, stop=True)
        cft = work.tile([S, CEXP], F32, tag="cft", bufs=2)
        nc.scalar.activation(cft[:], ps_th[:], AF.Sigmoid)
        nc.vector.tensor_scalar(cft[:], cft[:], sc, of, OP.mult, OP.add)
        cj_t = ctx_pool.tile([CTX, NBLK * S], F32, tag=f"cj{j}")
        for blk in range(NBLK):
            ps_c = ps_mm.tile([CTX, S], F32, tag="pse", bufs=3)
            nc.tensor.transpose(
                ps_c[:], cft[:, blk * CTX:(blk + 1) * CTX], w_idf[0:S, 0:S])
            nc.scalar.copy(cj_t[:, blk * S:(blk + 1) * S], ps_c[:])
        cj.append(cj_t)
    for i in range(2):  # fold dep-BN bias: b'_i = a_i*bnb + b_i
        for blk in range(NBLK):
            sl = slice(blk * S, (blk + 1) * S)
            nc.vector.scalar_tensor_tensor(
                cj[2 + i][:, sl], cj[i][:, sl], w_bnbd[:, blk:blk + 1],
                cj[2 + i][:, sl], OP.mult, OP.add)

    # per-channel big-branch selection: y1 = aB*e + bB on ACT; y2 = rho*y1+kappa
    NS = NBLK * S
    def ct_tile(tag):
        return ctx_pool.tile([CTX, NS], F32, tag=tag, name=tag)
    ab1 = ct_tile("ab1"); ab2 = ct_tile("ab2"); gsel = ct_tile("gsel")
    aB = ct_tile("aB"); bB = ct_tile("bB")
    rho = ct_tile("rho"); kap = ct_tile("kap")
    tA = ct_tile("tA"); tB = ct_tile("tB")
    nc.vector.tensor_scalar(ab1[:], cj[0][:], -1.0, None, OP.mult)
    nc.vector.tensor_tensor(ab1[:], cj[0][:], ab1[:], OP.max)
    nc.vector.tensor_scalar(ab2[:], cj[1][:], -1.0, None, OP.mult)
    nc.vector.tensor_tensor(ab2[:], cj[1][:], ab2[:], OP.max)
    nc.vector.tensor_tensor(gsel[:], ab1[:], ab2[:], OP.is_ge)
    nc.vector.tensor_tensor(tA[:], cj[0][:], cj[1][:], OP.subtract)
    nc.vector.tensor_tensor(tA[:], gsel[:], tA[:], OP.mult)
    nc.vector.tensor_tensor(aB[:], cj[1][:], tA[:], OP.add)
    nc.vector.tensor_tensor(tB[:], cj[2][:], cj[3][:], OP.subtract)
    nc.vector.tensor_tensor(tB[:], gsel[:], tB[:], OP.mult)
    nc.vector.tensor_tensor(bB[:], cj[3][:], tB[:], OP.add)
    nc.vector.tensor_tensor(tA[:], cj[0][:], cj[1][:], OP.add)
    nc.vector.tensor_tensor(tA[:], tA[:], aB[:], OP.subtract)   # a_small
    nc.vector.tensor_tensor(tB[:], cj[2][:], cj[3][:], OP.add)
    nc.vector.tensor_tensor(tB[:], tB[:], bB[:], OP.subtract)   # b_small
    nc.vector.reciprocal(kap[:], aB[:])
    nc.vector.tensor_tensor(rho[:], tA[:], kap[:], OP.mult)     # rho = aS/aB
    nc.vector.tensor_tensor(tA[:], rho[:], bB[:], OP.mult)
    nc.vector.tensor_tensor(kap[:], tB[:], tA[:], OP.subtract)  # kappa

    # CoordAtt gates: sigf f32 (TS scalar source), sigt bf16 (packed TT)
    sigf = ctx_pool.tile([CTX, NBLK * S * F], F32)
    sigt = ctx_pool.tile([CTX, NBLK * S * T], BF16)
    for blk in range(NBLK):
        ps_f = ps_mm.tile([CTX, S * F], F32, tag="pse", bufs=3)
        nc.tensor.matmul(ps_f[:], w_cvf[:, blk * CTX:(blk + 1) * CTX],
                         gc_b[:, 0:S * F], start=True, stop=True)
        nc.scalar.activation(sigf[:, blk * S * F:(blk + 1) * S * F], ps_f[:],
                             AF.Sigmoid, bias=w_bfb[:, blk:blk + 1], scale=1.0)
        ps_t2 = ps_mm.tile([CTX, S * T], F32, tag="pse", bufs=3)
        nc.tensor.matmul(ps_t2[:], w_cvt[:, blk * CTX:(blk + 1) * CTX],
                         gc_b[:, S * F:], start=True, stop=True)
        nc.scalar.activation(sigt[:, blk * S * T:(blk + 1) * S * T], ps_t2[:],
                             AF.Sigmoid, bias=w_btb[:, blk:blk + 1], scale=1.0)

    # ---------- Phase B : per-sample heavy pipeline ----------
    JW = 2 * NBLK * MB          # 1024: we row length (j, blk, i)

    for s in range(S):
        x_f8 = xpool.tile([CP, 2 * FT], F8, tag="xf8", bufs=2)
        nc.sync.dma_start(x_f8[:], xf8[s].rearrange("p a b -> p (a b)"))

        def acol(jr, k, parts):
            c0 = (jr * K + k) * S + s
            return att_b[0:parts, c0:c0 + 1]

        # --- mix expert weights ---
        we = work.tile([CP, JW], BF16, tag="we")
        nc.vector.tensor_scalar(we[:], w_expw[:, 0:JW], acol(0, 0, CP), None, OP.mult)
        for k in range(1, K - 1):
            nc.vector.scalar_tensor_tensor(
                we[:], w_expw[:, k * JW:(k + 1) * JW], acol(0, k, CP), we[:],
                OP.mult, OP.add)
        wef = work.tile([CP, JW], F8, tag="wef")
        nc.vector.scalar_tensor_tensor(
            wef[:], w_expw[:, (K - 1) * JW:K * JW], acol(0, K - 1, CP), we[:],
            OP.mult, OP.add)

        wd = work.tile([CTX, NBLK * 10], F32, tag="wd")
        nc.vector.tensor_scalar(wd[:], w_depw[:, 0:NBLK * 10], acol(1, 0, CTX),
                                None, OP.mult)
        for k in range(1, K):
            nc.vector.scalar_tensor_tensor(
                wd[:], w_depw[:, k * NBLK * 10:(k + 1) * NBLK * 10],
                acol(1, k, CTX), wd[:], OP.mult, OP.add)

        wp = work.tile([CTX, NBLK * COUT], BF16, tag="wp")
        nc.vector.tensor_scalar(wp[:], w_projw[:, 0:NBLK * COUT], acol(2, 0, CTX),
                                None, OP.mult)
        for k in range(1, K):
            nc.vector.scalar_tensor_tensor(
                wp[:], w_projw[:, k * NBLK * COUT:(k + 1) * NBLK * COUT],
                acol(2, k, CTX), wp[:], OP.mult, OP.add)

        zs = []
        for blk in range(NBLK):
            bs = blk * S + s
            # --- expansion: fp8 DoubleRow over channel pairs ---
            r_blk = work.tile([CTX, FT], BF16, tag="rb", bufs=2)
            for ch in range(NCH):
                ps_e = ps_mm.tile([MB, CHW], F32, tag="pse", bufs=3)
                lhs = AP(wef[:].tensor, blk * MB,
                         [[JW, CP], [NBLK * MB, 2], [1, MB]])
                rhs = AP(x_f8[:].tensor, ch * CHW,
                         [[2 * FT, CP], [FT, 2], [1, CHW]])
                nc.tensor.matmul(ps_e[:], lhs, rhs, start=True, stop=True,
                                 perf_mode=DR)
                nc.scalar.activation(r_blk[:, ch * CHW:(ch + 1) * CHW],
                                     ps_e[0:CTX, :], AF.Relu,
                                     bias=w_eb3[:, blk:blk + 1], scale=1.0)
            # --- hswish tail: bf16 on DVE, fp8 quantize via casting SWDGE DMA ---
            v_blk = work.tile([CTX, FT], BF16, tag="vb", bufs=2)
            nc.vector.tensor_scalar(v_blk[:], r_blk[:], 6.0, 1.0 / 6.0,
                                    OP.min, OP.mult)
            xe = xe_ring[bs % 2]
            xbp = xb_ring[bs % 2]
            xb3 = xbp[:, XOFF:XOFF + FP * TP].rearrange("p (f t) -> p f t", t=TP)
            xbi = xb3[:, 1:1 + F, TOFF:TOFF + T]
            nc.vector.tensor_scalar(
                xbi, r_blk[:].rearrange("p (f t) -> p f t", t=T),
                -3.0, None, OP.add)
            nc.vector.tensor_tensor(
                xbi, xbi, v_blk[:].rearrange("p (f t) -> p f t", t=T), OP.mult)
            nc.gpsimd.dma_start(xe[:], xbp[:])

            # --- diag weights for all 10 taps in one TT (GPSIMD) ---
            dg = work.tile([CTX, 10 * MB], F8, tag="dg", bufs=3)
            nc.gpsimd.tensor_tensor(
                dg[:].rearrange("p (t i) -> p t i", i=MB),
                w_i10[:].rearrange("p (t i) -> p t i", i=MB),
                wd[:, blk * 10:(blk + 1) * 10].unsqueeze(2)
                   .broadcast_to((CTX, 10, MB)),
                OP.mult)

            # --- depthwise: 5 fp8 DoubleRow passes per f-group ---
            z = zpool.tile([CTX, FT], BF16, tag=f"z{blk}")
            xe_pitch = xe[:].ap[0][0]
            dg_pitch = dg[:].ap[0][0]
            for g in range(NG):
                ps_d = ps_mm.tile([MB, GF * T], F32, tag="psd", bufs=3)
                for p, (df_a, dt_a, delta) in enumerate(PAIR_GEO):
                    base = XOFF + (1 + GF * g + df_a) * TP + TOFF + dt_a
                    rhs = AP(xe[:].tensor, base,
                             [[xe_pitch, CTX], [delta, 2], [TP, GF], [1, T]])
                    lhs = AP(dg[:].tensor, 2 * p * MB,
                             [[dg_pitch, CTX], [MB, 2], [1, MB]])
                    out_ap = AP(ps_d[:].tensor, 0,
                                [[ps_d[:].ap[0][0], MB], [T, GF], [1, T]])
                    nc.tensor.matmul(out_ap, lhs, rhs, start=(p == 0),
                                     stop=(p == NPAIR - 1), perf_mode=DR)
                # DyReLU: y1 on ACT (big branch), y2 = rho*y1+kappa on DVE
                sl = slice(g * GF * T, (g + 1) * GF * T)
                y1 = work.tile([CTX, GF * T], BF16, tag="y1", bufs=4)
                nc.scalar.activation(y1[:], ps_d[0:CTX, :], AF.Identity,
                                     bias=bB[:, bs:bs + 1],
                                     scale=aB[:, bs:bs + 1])
                y2 = work.tile([CTX, GF * T], BF16, tag="y2", bufs=4)
                nc.vector.tensor_scalar(y2[:], y1[:], rho[:, bs:bs + 1],
                                        kap[:, bs:bs + 1], OP.mult, OP.add)
                nc.vector.tensor_tensor(y1[:], y1[:], y2[:], OP.max)
                gt_v = sigt[:, bs * T:(bs + 1) * T].unsqueeze(1) \
                    .broadcast_to((CTX, GF, T))
                eng = nc.vector if (bs + g) % 2 == 0 else nc.gpsimd
                eng.tensor_tensor(
                    z[:, sl].rearrange("p (f t) -> p f t", t=T),
                    y1[:].rearrange("p (f t) -> p f t", t=T),
                    gt_v, OP.mult)
            zs.append(z)

        # --- projection per f-row (gf folded into lhsT) + residual ---

        wpgfs = []
        for blk in range(NBLK):
            # wpgf[c, (f, o)] = wp[c, o] * sigf[c, f]  (one 1x TT per blk)
            wpgf = work.tile([CTX, F * COUT], BF16, tag=f"wpgf{blk}", bufs=1)
            nc.vector.tensor_tensor(
                wpgf[:].rearrange("p (f o) -> p f o", o=COUT),
                wp[:, blk * COUT:(blk + 1) * COUT].unsqueeze(1)
                  .broadcast_to((CTX, F, COUT)),
                sigf[:, (blk * S + s) * F:(blk * S + s + 1) * F].unsqueeze(2)
                  .broadcast_to((CTX, F, COUT)),
                OP.mult)
            wpgfs.append(wpgf)
        xr = xpool.tile([CIN, FT], BF16, tag="xbf", bufs=2)
        nc.sync.dma_start(xr[:], xbf[s])
        for g2 in range(NG):
            ps_p = ps_mm.tile([COUT, GF * T], F32, tag="psp", bufs=2)
            for fi in range(GF):
                f = g2 * GF + fi
                for blk in range(NBLK):
                    nc.tensor.matmul(
                        ps_p[:, fi * T:(fi + 1) * T],
                        wpgfs[blk][:, f * COUT:(f + 1) * COUT],
                        zs[blk][:, f * T:(f + 1) * T],
                        start=(blk == 0), stop=False)
                nc.tensor.matmul(
                    ps_p[:, fi * T:(fi + 1) * T], w_i80[:],
                    xr[:, f * T:(f + 1) * T], start=False, stop=True)
            outs = work.tile([COUT, GF * T], F32, tag="outs", bufs=2)
            nc.scalar.activation(outs[:], ps_p[:], AF.Identity,
                                 bias=w_pbias[:, 0:1], scale=1.0)
            nc.sync.dma_start(yout[s, :, g2 * GF * T:(g2 + 1) * GF * T], outs[:])


def _host_prep(inputs):
    """Precompute packed/folded weight arrays (numpy, O(weights))."""
    p = {k: np.asarray(v, dtype=np.float32) for k, v in inputs.items()}
    inv_j = p["cg_joint_gamma"] / np.sqrt(p["cg_joint_var"] + EPS)
    sh_j = p["cg_joint_beta"] - p["cg_joint_mean"] * inv_j
    jlf = (p["cg_joint_w"].T * inv_j[None, :]) / T
    jlt = (p["cg_joint_w"].T * inv_j[None, :]) / F
    jb3 = (sh_j + 3.0)[:, None]

    cvf = np.ascontiguousarray(p["cg_convf_w"].T).astype(ml_dtypes.bfloat16)
    cvt = np.ascontiguousarray(p["cg_convt_w"].T).astype(ml_dtypes.bfloat16)
    bfb = np.ascontiguousarray(p["cg_convf_b"].reshape(NBLK, CTX).T)
    btb = np.ascontiguousarray(p["cg_convt_b"].reshape(NBLK, CTX).T)

    sc = 1.0 / ((F + T) * TEMP)
    arw0 = np.concatenate([p["exp_res_w"], p["dep_res_w"], p["proj_res_w"]], 0).T * sc
    arb0 = np.concatenate([p["exp_res_b"], p["dep_res_b"], p["proj_res_b"]]) / TEMP
    arw = np.ascontiguousarray(np.vstack([arw0, arb0[None, :]]))

    drw_r = p["dr_w"].reshape(CEXP, 2 * M, CTX).transpose(1, 0, 2)
    drw0 = drw_r.reshape(2 * M * CEXP, CTX).T / (F + T)
    drb_r = p["dr_b"].reshape(CEXP, 2 * M).T.reshape(-1)
    drw = np.ascontiguousarray(np.vstack([drw0, drb_r[None, :]])).astype(ml_dtypes.bfloat16)

    # expansion weights: BN-folded, padded to 128-col blocks, channel-pair
    # layout [40, K, 2, NBLK, 128]
    inv_e = p["exp_bn_gamma"] / np.sqrt(p["exp_bn_var"] + EPS)
    sh_e = p["exp_bn_beta"] - p["exp_bn_mean"] * inv_e
    ew = (p["exp_weight"] * inv_e[None, :, None]).transpose(0, 2, 1)  # [K,80,480]
    ewp = np.zeros((K, CIN, NBLK, MB), np.float32)
    ewp[:, :, :, 0:CTX] = ew.reshape(K, CIN, NBLK, CTX)
    expw = np.zeros((CP, K, 2, NBLK, MB), np.float32)
    for j in range(2):
        expw[:, :, j] = ewp[:, j * CP:(j + 1) * CP].transpose(1, 0, 2, 3)
    expw = np.ascontiguousarray(
        expw.reshape(CP, K * 2 * NBLK * MB)).astype(ml_dtypes.bfloat16)
    eb3 = np.ascontiguousarray((sh_e + 3.0).reshape(NBLK, CTX).T)

    # depthwise: BN-folded per-channel 3x3 in dt-parity tap order + zero pad
    inv_d = p["dep_bn_gamma"] / np.sqrt(p["dep_bn_var"] + EPS)
    sh_d = p["dep_bn_beta"] - p["dep_bn_mean"] * inv_d
    dw = (p["dep_weight"] * inv_d[None, :, None, None])  # [K, CEXP, 3, 3]
    dw10 = np.zeros((K, CEXP, 10), np.float32)
    for t, (df, dt) in enumerate(TAPS):
        dw10[:, :, t] = dw[:, :, df + 1, dt + 1]
    dw_b = dw10.reshape(K, NBLK, CTX, 10).transpose(2, 0, 1, 3)
    depw = np.ascontiguousarray(dw_b.reshape(CTX, K * NBLK * 10))
    bnbd = np.ascontiguousarray(sh_d.reshape(NBLK, CTX).T)

    inv_p = p["proj_bn_gamma"] / np.sqrt(p["proj_bn_var"] + EPS)
    sh_p = p["proj_bn_beta"] - p["proj_bn_mean"] * inv_p
    pw = p["proj_weight"] * inv_p[None, :, None]        # [K, 80, 480]
    pw_b = pw.transpose(2, 0, 1).reshape(NBLK, CTX, K, COUT).transpose(1, 2, 0, 3)
    projw = np.ascontiguousarray(
        pw_b.reshape(CTX, K * NBLK * COUT)).astype(ml_dtypes.bfloat16)
    pbias = sh_p[:, None]

    # 10 fp8 identity planes [120, 10*128] (plane t: delta_{i,c}; plane 9 zero)
    i10 = np.zeros((CTX, 10, MB), np.float32)
    for t in range(9):
        i10[np.arange(CTX), t, np.arange(CTX)] = 1.0
    i10 = i10.reshape(CTX, 10 * MB).astype(E4M3)

    identf = np.eye(128, dtype=np.float32)
    ident80 = np.eye(80, dtype=np.float32).astype(ml_dtypes.bfloat16)
    idp = np.zeros((CP, 2, MB), np.float32)
    for p_ in range(CP):
        for j_ in range(2):
            idp[p_, j_, p_ + CP * j_] = 1.0
    idp = idp.reshape(CP, 2 * MB).astype(E4M3)
    return dict(jlf=jlf, jlt=jlt, jb3=jb3, cvf=cvf, cvt=cvt, bfb=bfb, btb=btb,
                arw=arw, drw=drw, expw=expw, eb3=eb3,
                depw=depw, bnbd=bnbd, projw=projw, pbias=pbias,
                i10=i10, ident80=ident80, idp=idp, identf=identf,
                vtag=np.zeros((1, _VTAG), np.float32),
                ones1=np.ones((1, S), np.float32))


_BUILT = {}


def _build():
    if "nc" in _BUILT:
        return _BUILT["nc"]
    nc = bacc.Bacc("TRN2", target_bir_lowering=False, debug=False,
                   num_devices=NCORES)
    d = lambda n, s, dt=F32: nc.dram_tensor(n, list(s), dt, kind="ExternalInput").ap()
    io = [
        d("xbf", (S, CIN, FT), BF16),
        d("xf8", (S, CP, 2, FT), F8),
        d("jlf", (CIN, CTX)), d("jlt", (CIN, CTX)), d("jb3", (CTX, 1)),
        d("cvf", (CTX, CEXP), BF16), d("cvt", (CTX, CEXP), BF16),
        d("bfb", (CTX, NBLK)), d("btb", (CTX, NBLK)),
        d("arw", (CTX + 1, 3 * K)),
        d("drw", (CTX + 1, 2 * M * CEXP), BF16),
        d("expw", (CP, K * 2 * NBLK * MB), BF16), d("eb3", (CTX, NBLK)),
        d("depw", (CTX, K * NBLK * 10)), d("bnbd", (CTX, NBLK)),
        d("projw", (CTX, K * NBLK * COUT), BF16), d("pbias", (COUT, 1)),
        d("i10", (CTX, 10 * MB), F8),
        d("ident80", (COUT, COUT), BF16),
        d("idp", (CP, 2 * MB), F8),
        d("identf", (128, 128)), d("vtag", (1, _VTAG)),
        d("ones1", (1, S)),
        nc.dram_tensor("y", [S, COUT, FT], F32, kind="ExternalOutput").ap(),
    ]
    from contextlib import ExitStack
    with tile.TileContext(nc) as tc:
        with ExitStack() as es:
            _emit(tc, io, es)
    nc.compile()
    _BUILT["nc"] = nc
    return nc


def _purge_stale_neff_cache():
    """The neuronx compile cache can key on the HLO signature alone; purge
    defensively so a stale NEFF can never be loaded."""
    import shutil
    base = os.path.expanduser("~/.neuron-compile-cache")
    tag = os.path.join(base, f".dyblock_vtag_{_VTAG}")
    if os.path.exists(base) and not os.path.exists(tag):
        shutil.rmtree(base, ignore_errors=True)
        os.makedirs(base, exist_ok=True)
        open(tag, "w").close()


def make_in_maps(inputs):
    host = _host_prep(inputs)
    x = np.asarray(inputs["x"], dtype=np.float32).reshape(B, CIN, FT)
    xbf = x.astype(ml_dtypes.bfloat16)
    xf8 = np.stack([x[:, 0:CP], x[:, CP:CIN]], axis=2).astype(E4M3)  # [B,40,2,FT]
    in_maps = []
    for c in range(NCORES):
        m = {"xbf": np.ascontiguousarray(xbf[c * S:(c + 1) * S]),
             "xf8": np.ascontiguousarray(xf8[c * S:(c + 1) * S])}
        m.update(host)
        in_maps.append(m)
    return in_maps


def assemble_out(ys):
    out = np.concatenate(list(ys), axis=0)                # [B, COUT, FT]
    return np.ascontiguousarray(out.reshape(B, COUT, F, T))


def kernel(**inputs):
    _purge_stale_neff_cache()
    nc = _build()
    in_maps = make_in_maps(inputs)
    res = run_bass_kernel_spmd(nc, in_maps, list(range(NCORES)))
    return assemble_out([res.results[c]["y"] for c in range(NCORES)])


if __name__ == "__main__":
    import reference as ref
    inp = {k: np.asarray(v) for k, v in ref.setup_inputs().items()}
    got = kernel(**inp)
    from np_ref import forward_np
    exp = forward_np(inp)
    rel = np.abs(got - exp).max() / np.abs(exp).max()
    print("rel err vs np_ref:", rel)
